# revision 6
# baseline (speedup 1.0000x reference)
"""Causal self-attention (B=4, T=2048, D=1024, H=16) on 8 TRN2 NeuronCores.

Sharding: 2D (batch x head-group). Core c handles batch b = c//2 and head
group g = c%2 (8 heads, processed as 4 pairs).

v3 strategy (per core), building on v2's layout:
  - x is passed pre-transposed from host: xT [D, T].
  - Q/K projections produce qT/kT [128 local dims, T] with head pair 2p/2p+1
    stacked on partitions 0-63 / 64-127; the 1/sqrt(dh) scale is folded into
    the Wq weights on the host. The head-pair stacking makes the two score
    matmuls row-tiled (tile_position (0,0)/(64,0)) -> they run CONCURRENTLY
    on the PE (confirmed 3ns start deltas in traces).
  - V is projected directly into natural [token, dim] layout, stored bf16
    with a ones column per head so the PV matmul also accumulates the
    softmax denominator.
  - Scores are computed transposed: S^T [keys, queries]; causal masking is
    an accumulating PE matmul that adds -38 to masked positions BEFORE exp.
  - exp() runs without max-subtraction (scores ~N(0,1), fp32 exp safe);
    output P^T is bf16, feeding the PV matmul directly.
  - Softmax denominators: row 64 of the y PSUM tiles -> [2,512] tile,
    ONE fast-approx reciprocal, then ONE K=2 f32r matmul broadcasts both
    heads' 1/d to 64 partitions each (v2 used two K=1 matmuls + a
    [128,512] reciprocal).
  - o_proj consumes ynorm (bf16) as stationary; each core emits a partial
    [T, D] product over its 512 local head dims; host sums pairs.

v3 scheduling: the attention chain (score -> exp -> PV) is ACT-paced
(~1.1us/block) while the PE chain work is only ~0.65us/block.  v2 packed
projection/o_proj filler in 8-matmul bursts at PAIR boundaries, which
starved the Scalar engine ~70us total (measured).  v3 splits all filler
into single-matmul units and pops ~3 of them after EVERY 512-wide block
(fewer for narrower diagonal blocks), deliberately overfilling: PE total
work (~210us) exceeds ACT total (~157us), so the PE should never idle.
"""

import os
import sys
from collections import deque

import numpy as np

if not any(os.path.isdir(os.path.join(p, "concourse")) for p in sys.path):
    sys.path.insert(0, "/opt/trn_rl_repo")

import concourse.mybir as mybir
import concourse.tile as tile
from concourse import bacc
from concourse.bass_utils import run_bass_kernel_spmd

B, T, D, H, DH = 4, 2048, 1024, 16, 64
N_CORES = 8
GROUPS = 2          # head groups (tensor-parallel dim)
HPG = H // GROUPS   # heads per group/core
PAIRS = HPG // 2    # head pairs per core
NKB = T // 128      # 128-key blocks per batch
NQT = T // 512      # 512-query tiles per batch
VSTRIDE = NKB * 130 # vnat cols per pair: 16 blocks x [64 dims|1|64 dims|1]

F32 = mybir.dt.float32
F32R = mybir.dt.float32r
BF16 = mybir.dt.bfloat16


def build_nc():
    nc = bacc.Bacc("TRN2", target_bir_lowering=False, debug=False,
                   num_devices=N_CORES)
    xT = nc.dram_tensor("xT", [D, T], BF16, kind="ExternalInput").ap()
    wqR = nc.dram_tensor("wqR", [128, 4096], BF16, kind="ExternalInput").ap()
    wkR = nc.dram_tensor("wkR", [128, 4096], BF16, kind="ExternalInput").ap()
    wvT = nc.dram_tensor("wvT", [D, 512], BF16, kind="ExternalInput").ap()
    woT = nc.dram_tensor("woT", [512, D], BF16, kind="ExternalInput").ap()
    cpk = nc.dram_tensor("cpk", [128, 384], BF16, kind="ExternalInput").ap()
    onesb = nc.dram_tensor("onesb", [128, 128], BF16, kind="ExternalInput").ap()
    sel0 = nc.dram_tensor("sel0", [1, 128], BF16, kind="ExternalInput").ap()
    sel1 = nc.dram_tensor("sel1", [1, 128], BF16, kind="ExternalInput").ap()
    out = nc.dram_tensor("out", [T, D], F32, kind="ExternalOutput").ap()

    with tile.TileContext(nc) as tc:
        _body(tc, out, xT, wqR, wkR, wvT, woT, cpk, onesb, sel0, sel1)
    nc.compile()
    return nc


def _body(tc, out, xT, wqR, wkR, wvT, woT, cpk, onesb, sel0, sel1):
    nc = tc.nc
    from contextlib import ExitStack

    with ExitStack() as ctx:
        persist = ctx.enter_context(tc.tile_pool(name="persist", bufs=1))
        qT = persist.tile([128, PAIRS * T], BF16, tag="qT")
        kT = persist.tile([128, PAIRS * T], BF16, tag="kT")
        vnat = persist.tile([128, PAIRS * VSTRIDE], BF16, tag="vnat")
        ynorm = persist.tile([128, PAIRS * T], BF16, tag="ynorm")

        consts = ctx.enter_context(tc.tile_pool(name="consts", bufs=1))
        cpk_sb = consts.tile([128, 384], BF16, tag="cpk")
        nc.sync.dma_start(cpk_sb[:], cpk[:])
        mask2_sb = cpk_sb[:, 0:256]
        ident_sb = cpk_sb[:, 256:384]
        sel0_sb = consts.tile([1, 128], BF16, tag="sel0")
        sel1_sb = consts.tile([1, 128], BF16, tag="sel1")
        warm = consts.tile([128, 512], BF16, tag="warm")
        nc.vector.memset(warm[:], 0.0)

        wqkpool = ctx.enter_context(tc.tile_pool(name="wqk", bufs=1))
        wq_sb = wqkpool.tile([128, 4096], BF16, tag="wq")
        wk_sb = wqkpool.tile([128, 4096], BF16, tag="wk")
        wvpool = ctx.enter_context(tc.tile_pool(name="wv", bufs=1))
        wv_sb = wvpool.tile([128, 8 * 512], BF16, tag="wv")
        wopool = ctx.enter_context(tc.tile_pool(name="wo", bufs=1))
        wo_sb = []
        for p in range(PAIRS):
            wot = wopool.tile([128, 1024], BF16, tag=f"wo{p}")
            wo_sb.append(wot)

        xpool = ctx.enter_context(tc.tile_pool(name="xt", bufs=2))
        ppool = ctx.enter_context(tc.tile_pool(name="p", bufs=6))
        rpool = ctx.enter_context(tc.tile_pool(name="r", bufs=3))
        opool = ctx.enter_context(tc.tile_pool(name="osb", bufs=3))

        spool = ctx.enter_context(
            tc.tile_pool(name="s", bufs=2, space="PSUM"))
        ypool = ctx.enter_context(
            tc.tile_pool(name="y", bufs=1, space="PSUM"))
        shpool = ctx.enter_context(
            tc.tile_pool(name="sh", bufs=2, space="PSUM"))

        # ---------------- phase emitters -----------------------------
        x_half = [None, None]   # x_half[h] = list of 8 [128,1024] tiles

        def load_x(half, queue):
            tiles = []
            for c in range(8):
                xt = xpool.tile([128, 1024], BF16, tag=f"x{c}")
                queue.dma_start(xt[:], xT[c * 128:(c + 1) * 128,
                                           half * 1024:(half + 1) * 1024])
                tiles.append(xt)
            x_half[half] = tiles

        def prologue_dmas():
            # warm-up burst: dummy matmuls while DMAs stream, so the PE
            # HAM un-throttles before the first real projection matmul
            wps = shpool.tile([128, 512], F32, tag="ps")
            for i in range(16):
                nc.tensor.matmul(wps[:], lhsT=warm[:, 0:128], rhs=warm[:],
                                 start=True, stop=True)
            # wq strips on sync queue, x strips on gpsimd queue: parallel
            # issue so the first q-proj matmul can start after ~0.8MB
            for c in range(8):
                nc.sync.dma_start(wq_sb[:, c * 512:(c + 1) * 512],
                                  wqR[:, c * 512:(c + 1) * 512])
            load_x(0, nc.gpsimd)
            for c in range(8):
                nc.sync.dma_start(wk_sb[:, c * 512:(c + 1) * 512],
                                  wkR[:, c * 512:(c + 1) * 512])
            ones_view = vnat[:].rearrange("r (p k m x) -> r (p k m) x",
                                          p=PAIRS, k=NKB, m=2)[:, :, 64:65]
            nc.sync.dma_start(ones_view.squeeze(), onesb[:])
            for c in range(8):
                nc.sync.dma_start(wv_sb[:, c * 512:(c + 1) * 512],
                                  wvT[c * 128:(c + 1) * 128, :])
            for p in range(PAIRS):
                nc.sync.dma_start(wo_sb[p][:], woT[p * 128:(p + 1) * 128, :])
            nc.gpsimd.dma_start(sel0_sb[:], sel0[:])
            nc.gpsimd.dma_start(sel1_sb[:], sel1[:])
            load_x(1, nc.gpsimd)   # second half early; lands ~10us in

        def proj_units(half, sub):
            """Single-matmul emitters for one 512-token chunk: 8 q/k groups
            of (8 mm + 1 copy) and 4 v groups of (8 mm + 1 copy)."""
            xs = x_half[half]
            units = []
            for w_sb, dst in ((wq_sb, qT), (wk_sb, kT)):
                for p in range(PAIRS):
                    ps_box = [None]

                    def mk(w_sb=w_sb, p=p, ps_box=ps_box):
                        def mm(c, w_sb=w_sb, p=p, ps_box=ps_box):
                            if c == 0:
                                ps_box[0] = shpool.tile([128, 512], F32,
                                                        name="ps", tag="ps")
                            nc.tensor.matmul(
                                ps_box[0][:],
                                lhsT=(w_sb[:, c * 512 + p * 128:
                                             c * 512 + (p + 1) * 128]),
                                rhs=(xs[c][:, sub * 512:(sub + 1) * 512]),
                                start=(c == 0), stop=(c == 7))
                        return mm
                    mm = mk()
                    for c in range(8):
                        units.append(lambda c=c, mm=mm: mm(c))

                    def cp(dst=dst, p=p, ps_box=ps_box):
                        col0 = p * T + half * 1024 + sub * 512
                        nc.vector.tensor_copy(dst[:, col0:col0 + 512],
                                              ps_box[0][:])
                    units.append(cp)
            for tb in range(4):
                ps_box = [None]

                def mkv(tb=tb, ps_box=ps_box):
                    def mm(c, tb=tb, ps_box=ps_box):
                        if c == 0:
                            ps_box[0] = shpool.tile([128, 512], F32, name="ps", tag="ps")
                        tok0 = sub * 512 + tb * 128
                        nc.tensor.matmul(
                            ps_box[0][:],
                            lhsT=(xs[c][:, tok0:tok0 + 128]),
                            rhs=(wv_sb[:, c * 512:(c + 1) * 512]),
                            start=(c == 0), stop=(c == 7))
                    return mm
                mm = mkv()
                for c in range(8):
                    units.append(lambda c=c, mm=mm: mm(c))

                def cpv(tb=tb, ps_box=ps_box):
                    kb = half * 8 + sub * 4 + tb
                    srcv = ps_box[0][:].rearrange("r (p m x) -> r p m x",
                                                  p=PAIRS, m=2)
                    dstv = vnat[:].rearrange(
                        "r (p k m x) -> r p k m x",
                        p=PAIRS, k=NKB, m=2)[:, :, kb:kb + 1, :, 0:64]
                    nc.vector.tensor_copy(dstv.squeeze(2), srcv)
                units.append(cpv)
            return units

        def proj_chunk(half, sub):
            for u in proj_units(half, sub):
                u()

        def oproj_tt_units(tt):
            units = []
            for n in range(2):
                ps_box = [None]

                def mk(n=n, ps_box=ps_box):
                    def mm(p, n=n, ps_box=ps_box):
                        if p == 0:
                            ps_box[0] = shpool.tile([128, 512], F32, name="ps", tag="ps")
                        nc.tensor.matmul(
                            ps_box[0][:],
                            lhsT=(ynorm[:, p * T + tt * 128:
                                          p * T + tt * 128 + 128]),
                            rhs=(wo_sb[p][:, n * 512:(n + 1) * 512]),
                            start=(p == 0), stop=(p == PAIRS - 1))
                    return mm
                mm = mk()
                for p in range(PAIRS):
                    units.append(lambda p=p, mm=mm: mm(p))

                def fin(n=n, ps_box=ps_box):
                    osb = opool.tile([128, 512], F32, tag="osb")
                    nc.vector.tensor_copy(osb[:], ps_box[0][:])
                    nc.gpsimd.dma_start(
                        out[tt * 128:(tt + 1) * 128,
                            n * 512:(n + 1) * 512], osb[:])
                units.append(fin)
            return units

        pending = [None]

        def _normalize(p, qt, y0, y1):
            den0 = rpool.tile([1, 512], BF16, tag="den0")
            den1 = rpool.tile([1, 512], BF16, tag="den1")
            nc.vector.tensor_copy(den0[:], y0[64:65, :])
            nc.vector.tensor_copy(den1[:], y1[64:65, :])
            # broadcast raw denominators to 64 partitions each (bf16
            # K=1 matmuls: 215ns each vs 455ns for v2's fp32r), then one
            # fast-approx reciprocal on the full [128, 512] tile (DVE
            # cost is free-dim-bound, so this is as cheap as [1, 512])
            rbs = shpool.tile([128, 512], F32, tag="ps")
            nc.tensor.matmul(rbs[:], lhsT=sel0_sb[:],
                             rhs=den0[:], start=True, stop=False)
            nc.tensor.matmul(rbs[:], lhsT=sel1_sb[:],
                             rhs=den1[:], start=False, stop=True)
            rcp = rpool.tile([128, 512], F32, tag="rcp")
            nc.vector.reciprocal_approx_fast(out=rcp[:], in_=rbs[:])
            ycol = p * T + qt * 512
            nc.vector.tensor_mul(ynorm[0:64, ycol:ycol + 512],
                                 y0[0:64, :], rcp[0:64, :])
            nc.vector.tensor_mul(ynorm[64:128, ycol:ycol + 512],
                                 y1[0:64, :], rcp[64:128, :])

        def fill(n, fq):
            while n > 0 and fq:
                fq.popleft()()
                n -= 1

        def attn_qt(qt, fq):
            nkb = (qt + 1) * 4
            for p in range(PAIRS):
                fill(2, fq)
                y0 = ypool.tile([65, 512], F32, tag="y0")
                y1 = ypool.tile([65, 512], F32, tag="y1")
                for kb in range(nkb):
                    o = kb - qt * 4
                    scol = max(0, o * 128)
                    width = 512 - scol
                    qcol = p * T + qt * 512 + scol
                    kcol = p * T + kb * 128
                    vbase = p * VSTRIDE + kb * 130
                    # both heads' scores in one 2-bank PSUM tile so a
                    # single ACT instruction exponentiates both; the two
                    # matmuls are row-tiled (partitions 0-63 / 64-127)
                    # and execute concurrently on the PE
                    s01 = spool.tile([128, 1024], F32, tag="s01")
                    nc.tensor.matmul(
                        s01[:, 0:width],
                        lhsT=(kT[0:64, kcol:kcol + 128]),
                        rhs=(qT[0:64, qcol:qcol + width]),
                        start=True, stop=(o < 0))
                    nc.tensor.matmul(
                        s01[:, 512:512 + width],
                        lhsT=(kT[64:128, kcol:kcol + 128]),
                        rhs=(qT[64:128, qcol:qcol + width]),
                        start=True, stop=(o < 0))
                    if o >= 0:
                        # causal mask: accumulate -38 into masked positions
                        # of the diagonal 128-col chunk (both heads in one
                        # N=256 matmul)
                        mview = s01[:].rearrange("r (h x) -> r h x",
                                                 h=2)[:, :, 0:128]
                        nc.tensor.matmul(
                            mview, lhsT=ident_sb,
                            rhs=mask2_sb.rearrange("r (h x) -> r h x",
                                                      h=2),
                            start=False, stop=True)
                    p01 = ppool.tile([128, 1024], BF16, tag="p01")
                    sview = s01[:].rearrange("r (h x) -> r h x",
                                             h=2)[:, :, 0:width]
                    pview = p01[:].rearrange("r (h x) -> r h x",
                                             h=2)[:, :, 0:width]
                    nc.scalar.activation(
                        pview, sview, mybir.ActivationFunctionType.Exp)
                    nc.tensor.matmul(
                        y0[:, scol:512],
                        lhsT=(vnat[:, vbase:vbase + 65]),
                        rhs=(p01[:, 0:width]),
                        start=(kb == 0), stop=(kb == nkb - 1))
                    nc.tensor.matmul(
                        y1[:, scol:512],
                        lhsT=(vnat[:, vbase + 65:vbase + 130]),
                        rhs=(p01[:, 512:512 + width]),
                        start=(kb == 0), stop=(kb == nkb - 1))
                    if kb == 1 and pending[0] is not None:
                        pending[0]()
                        pending[0] = None
                    # fine-grained filler: keep the PE fed during the
                    # ACT-bound exp of this block
                    fill(3 if width >= 512 else (2 if width >= 256 else 1),
                         fq)
                if pending[0] is not None:
                    pending[0]()
                pending[0] = (lambda p=p, qt=qt, y0=y0, y1=y1:
                              _normalize(p, qt, y0, y1))

        def flush_pending():
            if pending[0] is not None:
                pending[0]()
                pending[0] = None

        # ---------------- emission order -----------------------------
        def drain(fq):
            # qt i+1's chain reads tiles written by qt i's filler chunk;
            # those writes must be EMITTED before the reads or no sem
            # orders them.  Drain leftovers at every qt boundary.
            while fq:
                fq.popleft()()

        prologue_dmas()
        proj_chunk(0, 0)
        fq = deque(proj_units(0, 1))
        attn_qt(0, fq)
        drain(fq)                # qt1 needs all of chunk(0,1)
        fq.extend(proj_units(1, 0))
        attn_qt(1, fq)
        drain(fq)                # qt2 needs all of chunk(1,0)
        fq.extend(proj_units(1, 1))
        flush_pending()          # ynorm qt0/qt1 complete for oproj filler
        for tt in range(8):
            fq.extend(oproj_tt_units(tt))
        attn_qt(2, fq)
        drain(fq)                # qt3 needs all of chunk(1,1)
        flush_pending()
        for tt in range(8, 12):
            fq.extend(oproj_tt_units(tt))
        attn_qt(3, fq)
        flush_pending()
        drain(fq)
        for tt in range(12, 16):
            for u in oproj_tt_units(tt):
                u()


def shard_inputs(x, Wq, Wk, Wv, Wo):
    """Returns in_maps for cores 0..7 (core c: batch c//2, group c%2)."""
    import ml_dtypes
    x = np.ascontiguousarray(np.asarray(x, np.float32))
    mask1 = np.zeros((128, 128), np.float32)
    for r in range(128):
        mask1[r, :r] = -38.0  # S^T[key r, query j]: masked iff j < r
    mask = np.ascontiguousarray(np.concatenate([mask1, mask1], axis=1))
    ident = np.eye(128, dtype=np.float32)
    sel0 = np.zeros((1, 128), np.float32); sel0[0, :64] = 1.0
    sel1 = np.zeros((1, 128), np.float32); sel1[0, 64:] = 1.0
    in_maps = []
    perms = []
    for g in range(GROUPS):
        perm = np.array([(g * HPG + 2 * p + (q >= 64)) * 64 + (q % 64)
                         for p in range(PAIRS) for q in range(128)])
        perms.append(perm)
    w_cache = {}
    qscale = 1.0 / np.sqrt(DH)
    for g in range(GROUPS):
        perm = perms[g]
        wqT = (np.asarray(Wq, np.float32).T * qscale)[:, perm]
        wkT = np.asarray(Wk, np.float32).T[:, perm]
        # [r, c*512 + p*128 + o] = wT[c*128 + r, p*128 + o]
        def _re(wT):
            w4 = wT.reshape(8, 128, 4, 128)        # [c, r, p, o]
            return np.ascontiguousarray(
                w4.transpose(1, 0, 2, 3).reshape(128, 4096))
        w_cache[g] = {
            "wqR": _re(wqT).astype(ml_dtypes.bfloat16),
            "wkR": _re(wkT).astype(ml_dtypes.bfloat16),
            "wvT": np.ascontiguousarray(
                np.asarray(Wv, np.float32).T[:, perm]).astype(
                    ml_dtypes.bfloat16),
            "woT": np.ascontiguousarray(
                np.asarray(Wo, np.float32).T[perm, :]).astype(
                    ml_dtypes.bfloat16),
        }
    for c in range(N_CORES):
        b, g = c // 2, c % 2
        in_maps.append({
            "xT": np.ascontiguousarray(x[b].T).astype(ml_dtypes.bfloat16),
            "cpk": np.ascontiguousarray(
                np.concatenate([mask, ident], axis=1)).astype(
                    ml_dtypes.bfloat16),
            "onesb": np.ones((128, 128), ml_dtypes.bfloat16),
            "sel0": sel0.astype(ml_dtypes.bfloat16),
            "sel1": sel1.astype(ml_dtypes.bfloat16),
            **w_cache[g],
        })
    return in_maps


def kernel(x, Wq, Wk, Wv, Wo):
    nc = build_nc()
    in_maps = shard_inputs(x, Wq, Wk, Wv, Wo)
    res = run_bass_kernel_spmd(nc, in_maps, list(range(N_CORES)))
    out = np.empty((B, T, D), np.float32)
    for b in range(B):
        out[b] = res.results[2 * b]["out"] + res.results[2 * b + 1]["out"]
    return out


# revision 9
# speedup vs baseline: 1.0527x; 1.0527x over previous
"""Causal self-attention (B=4, T=2048, D=1024, H=16) on 8 TRN2 NeuronCores.

Sharding: 2D (batch x head-group). Core c handles batch b = c//2 and head
group g = c%2 (8 heads, processed as 4 pairs).

v3 strategy (per core), building on v2's layout:
  - x is passed pre-transposed from host: xT [D, T].
  - Q/K projections produce qT/kT [128 local dims, T] with head pair 2p/2p+1
    stacked on partitions 0-63 / 64-127; the 1/sqrt(dh) scale is folded into
    the Wq weights on the host. The head-pair stacking makes the two score
    matmuls row-tiled (tile_position (0,0)/(64,0)) -> they run CONCURRENTLY
    on the PE (confirmed 3ns start deltas in traces).
  - V is projected directly into natural [token, dim] layout, stored bf16
    with a ones column per head so the PV matmul also accumulates the
    softmax denominator.
  - Scores are computed transposed: S^T [keys, queries]; causal masking is
    an accumulating PE matmul that adds -38 to masked positions BEFORE exp.
  - exp() runs without max-subtraction (scores ~N(0,1), fp32 exp safe);
    output P^T is bf16, feeding the PV matmul directly.
  - Softmax denominators: row 64 of the y PSUM tiles -> [2,512] tile,
    ONE fast-approx reciprocal, then ONE K=2 f32r matmul broadcasts both
    heads' 1/d to 64 partitions each (v2 used two K=1 matmuls + a
    [128,512] reciprocal).
  - o_proj consumes ynorm (bf16) as stationary; each core emits a partial
    [T, D] product over its 512 local head dims; host sums pairs.

v3 scheduling: the attention chain (score -> exp -> PV) is ACT-paced
(~1.1us/block) while the PE chain work is only ~0.65us/block.  v2 packed
projection/o_proj filler in 8-matmul bursts at PAIR boundaries, which
starved the Scalar engine ~70us total (measured).  v3 splits all filler
into single-matmul units and pops ~3 of them after EVERY 512-wide block
(fewer for narrower diagonal blocks), deliberately overfilling: PE total
work (~210us) exceeds ACT total (~157us), so the PE should never idle.
"""

import os
import sys
from collections import deque

import numpy as np

if not any(os.path.isdir(os.path.join(p, "concourse")) for p in sys.path):
    sys.path.insert(0, "/opt/trn_rl_repo")

import concourse.mybir as mybir
import concourse.tile as tile
from concourse import bacc
from concourse.bass_utils import run_bass_kernel_spmd

B, T, D, H, DH = 4, 2048, 1024, 16, 64
N_CORES = 8
GROUPS = 2          # head groups (tensor-parallel dim)
HPG = H // GROUPS   # heads per group/core
PAIRS = HPG // 2    # head pairs per core
NKB = T // 128      # 128-key blocks per batch
NQT = T // 512      # 512-query tiles per batch
VSTRIDE = NKB * 130 # vnat cols per pair: 16 blocks x [64 dims|1|64 dims|1]

F32 = mybir.dt.float32
F32R = mybir.dt.float32r
BF16 = mybir.dt.bfloat16


def build_nc():
    nc = bacc.Bacc("TRN2", target_bir_lowering=False, debug=False,
                   num_devices=N_CORES)
    xT = nc.dram_tensor("xT", [D, T], BF16, kind="ExternalInput").ap()
    wqR = nc.dram_tensor("wqR", [128, 4096], BF16, kind="ExternalInput").ap()
    wkR = nc.dram_tensor("wkR", [128, 4096], BF16, kind="ExternalInput").ap()
    wvT = nc.dram_tensor("wvT", [D, 512], BF16, kind="ExternalInput").ap()
    woT = nc.dram_tensor("woT", [512, D], BF16, kind="ExternalInput").ap()
    cpk = nc.dram_tensor("cpk", [128, 384], BF16, kind="ExternalInput").ap()
    onesb = nc.dram_tensor("onesb", [128, 128], BF16, kind="ExternalInput").ap()
    sel0 = nc.dram_tensor("sel0", [1, 128], BF16, kind="ExternalInput").ap()
    sel1 = nc.dram_tensor("sel1", [1, 128], BF16, kind="ExternalInput").ap()
    out = nc.dram_tensor("out", [T, D], F32, kind="ExternalOutput").ap()

    with tile.TileContext(nc) as tc:
        _body(tc, out, xT, wqR, wkR, wvT, woT, cpk, onesb, sel0, sel1)
    nc.compile()
    return nc


def _body(tc, out, xT, wqR, wkR, wvT, woT, cpk, onesb, sel0, sel1):
    nc = tc.nc
    from contextlib import ExitStack

    with ExitStack() as ctx:
        persist = ctx.enter_context(tc.tile_pool(name="persist", bufs=1))
        qT = persist.tile([128, PAIRS * T], BF16, tag="qT")
        kT = persist.tile([128, PAIRS * T], BF16, tag="kT")
        vnat = persist.tile([128, PAIRS * VSTRIDE], BF16, tag="vnat")
        ynorm = persist.tile([128, PAIRS * T], BF16, tag="ynorm")

        consts = ctx.enter_context(tc.tile_pool(name="consts", bufs=1))
        cpk_sb = consts.tile([128, 384], BF16, tag="cpk")
        nc.sync.dma_start(cpk_sb[:], cpk[:])
        mask2_sb = cpk_sb[:, 0:256]
        ident_sb = cpk_sb[:, 256:384]
        sel0_sb = consts.tile([1, 128], BF16, tag="sel0")
        sel1_sb = consts.tile([1, 128], BF16, tag="sel1")
        warm = consts.tile([128, 512], BF16, tag="warm")
        nc.vector.memset(warm[:], 0.0)

        wqkpool = ctx.enter_context(tc.tile_pool(name="wqk", bufs=1))
        wq_sb = wqkpool.tile([128, 4096], BF16, tag="wq")
        wk_sb = wqkpool.tile([128, 4096], BF16, tag="wk")
        wvpool = ctx.enter_context(tc.tile_pool(name="wv", bufs=1))
        wv_sb = wvpool.tile([128, 8 * 512], BF16, tag="wv")
        wopool = ctx.enter_context(tc.tile_pool(name="wo", bufs=1))
        wo_sb = []
        for p in range(PAIRS):
            wot = wopool.tile([128, 1024], BF16, tag=f"wo{p}")
            wo_sb.append(wot)

        xpool = ctx.enter_context(tc.tile_pool(name="xt", bufs=2))
        ppool = ctx.enter_context(tc.tile_pool(name="p", bufs=6))
        rpool = ctx.enter_context(tc.tile_pool(name="r", bufs=3))
        opool = ctx.enter_context(tc.tile_pool(name="osb", bufs=3))

        spool = ctx.enter_context(
            tc.tile_pool(name="s", bufs=2, space="PSUM"))
        ypool = ctx.enter_context(
            tc.tile_pool(name="y", bufs=1, space="PSUM"))
        shpool = ctx.enter_context(
            tc.tile_pool(name="sh", bufs=2, space="PSUM"))

        # ---------------- phase emitters -----------------------------
        x_half = [None, None]   # x_half[h] = list of 8 [128,1024] tiles

        def load_x(half, queue):
            tiles = []
            for c in range(8):
                xt = xpool.tile([128, 1024], BF16, tag=f"x{c}")
                queue.dma_start(xt[:], xT[c * 128:(c + 1) * 128,
                                           half * 1024:(half + 1) * 1024])
                tiles.append(xt)
            x_half[half] = tiles

        def prologue_dmas():
            # warm-up burst: dummy matmuls while DMAs stream, so the PE
            # HAM un-throttles before the first real projection matmul
            wps = shpool.tile([128, 512], F32, tag="ps")
            for i in range(16):
                nc.tensor.matmul(wps[:], lhsT=warm[:, 0:128], rhs=warm[:],
                                 start=True, stop=True)
            # DMA issue is the bottleneck (~650ns/descriptor per queue):
            # spread the 8.3MB of inputs over FOUR queues so everything
            # lands by ~12us (v2/v3a serialized wq+wk+ones+wv+wo on sync;
            # wv only landed ~40us in, stalling v-proj units).
            for c in range(8):
                nc.sync.dma_start(wq_sb[:, c * 512:(c + 1) * 512],
                                  wqR[:, c * 512:(c + 1) * 512])
            load_x(0, nc.gpsimd)
            for c in range(8):
                nc.scalar.dma_start(wk_sb[:, c * 512:(c + 1) * 512],
                                    wkR[:, c * 512:(c + 1) * 512])
            ones_view = vnat[:].rearrange("r (p k m x) -> r (p k m) x",
                                          p=PAIRS, k=NKB, m=2)[:, :, 64:65]
            nc.scalar.dma_start(ones_view.squeeze(), onesb[:])
            for c in range(8):
                nc.scalar.dma_start(wv_sb[:, c * 512:(c + 1) * 512],
                                    wvT[c * 128:(c + 1) * 128, :])
            for p in range(PAIRS):
                nc.sync.dma_start(wo_sb[p][:], woT[p * 128:(p + 1) * 128, :])
            nc.gpsimd.dma_start(sel0_sb[:], sel0[:])
            nc.gpsimd.dma_start(sel1_sb[:], sel1[:])
            load_x(1, nc.gpsimd)   # second half early; lands ~18us in

        def proj_units(half, sub):
            """Single-matmul emitters for one 512-token chunk: 8 q/k groups
            of (8 mm + 1 copy) and 4 v groups of (8 mm + 1 copy)."""
            xs = x_half[half]
            units = []
            for w_sb, dst in ((wq_sb, qT), (wk_sb, kT)):
                for p in range(PAIRS):
                    ps_box = [None]

                    def mk(w_sb=w_sb, p=p, ps_box=ps_box):
                        def mm(c, w_sb=w_sb, p=p, ps_box=ps_box):
                            if c == 0:
                                ps_box[0] = shpool.tile([128, 512], F32,
                                                        name="ps", tag="ps")
                            nc.tensor.matmul(
                                ps_box[0][:],
                                lhsT=(w_sb[:, c * 512 + p * 128:
                                             c * 512 + (p + 1) * 128]),
                                rhs=(xs[c][:, sub * 512:(sub + 1) * 512]),
                                start=(c == 0), stop=(c == 7))
                        return mm
                    mm = mk()
                    for c in range(8):
                        units.append(lambda c=c, mm=mm: mm(c))

                    def cp(dst=dst, p=p, ps_box=ps_box):
                        col0 = p * T + half * 1024 + sub * 512
                        nc.vector.tensor_copy(dst[:, col0:col0 + 512],
                                              ps_box[0][:])
                    units.append(cp)
            for tb in range(4):
                ps_box = [None]

                def mkv(tb=tb, ps_box=ps_box):
                    def mm(c, tb=tb, ps_box=ps_box):
                        if c == 0:
                            ps_box[0] = shpool.tile([128, 512], F32, name="ps", tag="ps")
                        tok0 = sub * 512 + tb * 128
                        nc.tensor.matmul(
                            ps_box[0][:],
                            lhsT=(xs[c][:, tok0:tok0 + 128]),
                            rhs=(wv_sb[:, c * 512:(c + 1) * 512]),
                            start=(c == 0), stop=(c == 7))
                    return mm
                mm = mkv()
                for c in range(8):
                    units.append(lambda c=c, mm=mm: mm(c))

                def cpv(tb=tb, ps_box=ps_box):
                    kb = half * 8 + sub * 4 + tb
                    srcv = ps_box[0][:].rearrange("r (p m x) -> r p m x",
                                                  p=PAIRS, m=2)
                    dstv = vnat[:].rearrange(
                        "r (p k m x) -> r p k m x",
                        p=PAIRS, k=NKB, m=2)[:, :, kb:kb + 1, :, 0:64]
                    nc.vector.tensor_copy(dstv.squeeze(2), srcv)
                units.append(cpv)
            return units

        def proj_chunk(half, sub):
            for u in proj_units(half, sub):
                u()

        def oproj_tt_units(tt):
            units = []
            for n in range(2):
                ps_box = [None]

                def mk(n=n, ps_box=ps_box):
                    def mm(p, n=n, ps_box=ps_box):
                        if p == 0:
                            ps_box[0] = shpool.tile([128, 512], F32, name="ps", tag="ps")
                        nc.tensor.matmul(
                            ps_box[0][:],
                            lhsT=(ynorm[:, p * T + tt * 128:
                                          p * T + tt * 128 + 128]),
                            rhs=(wo_sb[p][:, n * 512:(n + 1) * 512]),
                            start=(p == 0), stop=(p == PAIRS - 1))
                    return mm
                mm = mk()
                for p in range(PAIRS):
                    units.append(lambda p=p, mm=mm: mm(p))

                def fin(n=n, ps_box=ps_box):
                    osb = opool.tile([128, 512], F32, tag="osb")
                    nc.vector.tensor_copy(osb[:], ps_box[0][:])
                    nc.gpsimd.dma_start(
                        out[tt * 128:(tt + 1) * 128,
                            n * 512:(n + 1) * 512], osb[:])
                units.append(fin)
            return units

        pending = [None]

        def _normalize(p, qt, y0, y1):
            den0 = rpool.tile([1, 512], BF16, tag="den0")
            den1 = rpool.tile([1, 512], BF16, tag="den1")
            nc.vector.tensor_copy(den0[:], y0[64:65, :])
            nc.vector.tensor_copy(den1[:], y1[64:65, :])
            # broadcast raw denominators to 64 partitions each (bf16
            # K=1 matmuls: 215ns each vs 455ns for v2's fp32r), then one
            # fast-approx reciprocal on the full [128, 512] tile (DVE
            # cost is free-dim-bound, so this is as cheap as [1, 512])
            rbs = shpool.tile([128, 512], F32, tag="ps")
            nc.tensor.matmul(rbs[:], lhsT=sel0_sb[:],
                             rhs=den0[:], start=True, stop=False)
            nc.tensor.matmul(rbs[:], lhsT=sel1_sb[:],
                             rhs=den1[:], start=False, stop=True)
            rcp = rpool.tile([128, 512], F32, tag="rcp")
            nc.vector.reciprocal_approx_fast(out=rcp[:], in_=rbs[:])
            ycol = p * T + qt * 512
            nc.vector.tensor_mul(ynorm[0:64, ycol:ycol + 512],
                                 y0[0:64, :], rcp[0:64, :])
            nc.vector.tensor_mul(ynorm[64:128, ycol:ycol + 512],
                                 y1[0:64, :], rcp[64:128, :])

        def fill(n, fq):
            while n > 0 and fq:
                fq.popleft()()
                n -= 1

        def attn_qt(qt, fq, rate=3):
            nkb = (qt + 1) * 4
            for p in range(PAIRS):
                fill(2, fq)
                y0 = ypool.tile([65, 512], F32, tag="y0")
                y1 = ypool.tile([65, 512], F32, tag="y1")
                for kb in range(nkb):
                    o = kb - qt * 4
                    scol = max(0, o * 128)
                    width = 512 - scol
                    qcol = p * T + qt * 512 + scol
                    kcol = p * T + kb * 128
                    vbase = p * VSTRIDE + kb * 130
                    # both heads' scores in one 2-bank PSUM tile so a
                    # single ACT instruction exponentiates both; the two
                    # matmuls are row-tiled (partitions 0-63 / 64-127)
                    # and execute concurrently on the PE
                    s01 = spool.tile([128, 1024], F32, tag="s01")
                    nc.tensor.matmul(
                        s01[:, 0:width],
                        lhsT=(kT[0:64, kcol:kcol + 128]),
                        rhs=(qT[0:64, qcol:qcol + width]),
                        start=True, stop=(o < 0))
                    nc.tensor.matmul(
                        s01[:, 512:512 + width],
                        lhsT=(kT[64:128, kcol:kcol + 128]),
                        rhs=(qT[64:128, qcol:qcol + width]),
                        start=True, stop=(o < 0))
                    if o >= 0:
                        # causal mask: accumulate -38 into masked positions
                        # of the diagonal 128-col chunk (both heads in one
                        # N=256 matmul)
                        mview = s01[:].rearrange("r (h x) -> r h x",
                                                 h=2)[:, :, 0:128]
                        nc.tensor.matmul(
                            mview, lhsT=ident_sb,
                            rhs=mask2_sb.rearrange("r (h x) -> r h x",
                                                      h=2),
                            start=False, stop=True)
                    p01 = ppool.tile([128, 1024], BF16, tag="p01")
                    sview = s01[:].rearrange("r (h x) -> r h x",
                                             h=2)[:, :, 0:width]
                    pview = p01[:].rearrange("r (h x) -> r h x",
                                             h=2)[:, :, 0:width]
                    nc.scalar.activation(
                        pview, sview, mybir.ActivationFunctionType.Exp)
                    nc.tensor.matmul(
                        y0[:, scol:512],
                        lhsT=(vnat[:, vbase:vbase + 65]),
                        rhs=(p01[:, 0:width]),
                        start=(kb == 0), stop=(kb == nkb - 1))
                    nc.tensor.matmul(
                        y1[:, scol:512],
                        lhsT=(vnat[:, vbase + 65:vbase + 130]),
                        rhs=(p01[:, 512:512 + width]),
                        start=(kb == 0), stop=(kb == nkb - 1))
                    if kb == 1 and pending[0] is not None:
                        pending[0]()
                        pending[0] = None
                    # fine-grained filler: keep the PE fed during the
                    # ACT-bound exp of this block
                    fill(rate if width >= 512 else (2 if width >= 256 else 1),
                         fq)
                if pending[0] is not None:
                    pending[0]()
                pending[0] = (lambda p=p, qt=qt, y0=y0, y1=y1:
                              _normalize(p, qt, y0, y1))

        def flush_pending():
            if pending[0] is not None:
                pending[0]()
                pending[0] = None

        # ---------------- emission order -----------------------------
        def drain(fq):
            # qt i+1's chain reads tiles written by qt i's filler chunk;
            # those writes must be EMITTED before the reads or no sem
            # orders them.  Drain leftovers at every qt boundary.
            while fq:
                fq.popleft()()

        prologue_dmas()
        proj_chunk(0, 0)
        fq = deque(proj_units(0, 1))
        attn_qt(0, fq, rate=3)
        drain(fq)                # qt1 needs all of chunk(0,1)
        fq.extend(proj_units(1, 0))
        attn_qt(1, fq, rate=3)
        drain(fq)                # qt2 needs all of chunk(1,0)
        fq.extend(proj_units(1, 1))
        attn_qt(2, fq, rate=2)
        drain(fq)                # qt3 needs all of chunk(1,1)
        flush_pending()          # ynorm qt0-2 complete for oproj filler
        for tt in range(12):
            fq.extend(oproj_tt_units(tt))
        attn_qt(3, fq, rate=2)
        flush_pending()
        drain(fq)
        for tt in range(12, 16):
            for u in oproj_tt_units(tt):
                u()


def shard_inputs(x, Wq, Wk, Wv, Wo):
    """Returns in_maps for cores 0..7 (core c: batch c//2, group c%2)."""
    import ml_dtypes
    x = np.ascontiguousarray(np.asarray(x, np.float32))
    mask1 = np.zeros((128, 128), np.float32)
    for r in range(128):
        mask1[r, :r] = -38.0  # S^T[key r, query j]: masked iff j < r
    mask = np.ascontiguousarray(np.concatenate([mask1, mask1], axis=1))
    ident = np.eye(128, dtype=np.float32)
    sel0 = np.zeros((1, 128), np.float32); sel0[0, :64] = 1.0
    sel1 = np.zeros((1, 128), np.float32); sel1[0, 64:] = 1.0
    in_maps = []
    perms = []
    for g in range(GROUPS):
        perm = np.array([(g * HPG + 2 * p + (q >= 64)) * 64 + (q % 64)
                         for p in range(PAIRS) for q in range(128)])
        perms.append(perm)
    w_cache = {}
    qscale = 1.0 / np.sqrt(DH)
    for g in range(GROUPS):
        perm = perms[g]
        wqT = (np.asarray(Wq, np.float32).T * qscale)[:, perm]
        wkT = np.asarray(Wk, np.float32).T[:, perm]
        # [r, c*512 + p*128 + o] = wT[c*128 + r, p*128 + o]
        def _re(wT):
            w4 = wT.reshape(8, 128, 4, 128)        # [c, r, p, o]
            return np.ascontiguousarray(
                w4.transpose(1, 0, 2, 3).reshape(128, 4096))
        w_cache[g] = {
            "wqR": _re(wqT).astype(ml_dtypes.bfloat16),
            "wkR": _re(wkT).astype(ml_dtypes.bfloat16),
            "wvT": np.ascontiguousarray(
                np.asarray(Wv, np.float32).T[:, perm]).astype(
                    ml_dtypes.bfloat16),
            "woT": np.ascontiguousarray(
                np.asarray(Wo, np.float32).T[perm, :]).astype(
                    ml_dtypes.bfloat16),
        }
    for c in range(N_CORES):
        b, g = c // 2, c % 2
        in_maps.append({
            "xT": np.ascontiguousarray(x[b].T).astype(ml_dtypes.bfloat16),
            "cpk": np.ascontiguousarray(
                np.concatenate([mask, ident], axis=1)).astype(
                    ml_dtypes.bfloat16),
            "onesb": np.ones((128, 128), ml_dtypes.bfloat16),
            "sel0": sel0.astype(ml_dtypes.bfloat16),
            "sel1": sel1.astype(ml_dtypes.bfloat16),
            **w_cache[g],
        })
    return in_maps


def kernel(x, Wq, Wk, Wv, Wo):
    nc = build_nc()
    in_maps = shard_inputs(x, Wq, Wk, Wv, Wo)
    res = run_bass_kernel_spmd(nc, in_maps, list(range(N_CORES)))
    out = np.empty((B, T, D), np.float32)
    for b in range(B):
        out[b] = res.results[2 * b]["out"] + res.results[2 * b + 1]["out"]
    return out
